# revision 37
# baseline (speedup 1.0000x reference)
"""ALIGNN edge-gated message passing on 8 Trainium2 NeuronCores.

Strategy: edges partitioned by dst-block across cores (no collectives).
Each core receives host-prepared, per-core data:
  - a compacted node table (only nodes referenced as src by its edges)
  - its own 98 node-blocks (128 nodes each) in load-balanced "slot" order
  - its edge slice in canonical (slot, window, tile) order
Device work per core:
  phase A : PE matmuls node_feats -> T1=[e_src+bias | Bh] (f32, compact rows)
            and T2x=[e_dst | x_lin] (bf16, own blocks)
  phase B : per (slot,window) group of 128-edge tiles: dma_gather T1 rows by
            src (int16 windows), one-hot(dst_local) via batched is_equal,
            m = ef@W_eg + onehot.T@T2blk (+ gathered), sigma = sigmoid(m),
            segment-sum via one-hot matmul accumulation in PSUM;
            LN sqrt batched over deferred groups of 32 tiles, then
            y = ef + silu(LN(m)) written edge-major
  phase D : per slot: xpre = x_lin + ssh/(ss+1e-6); final phase applies
            LN + silu + residual for all slots with batched stats
"""
import sys

if '/opt/trn_rl_repo' not in sys.path:
    sys.path.insert(0, '/opt/trn_rl_repo')

import numpy as np
import ml_dtypes

BF16 = ml_dtypes.bfloat16
H = 96
LN_EPS = 1e-5
NCORES = 8
P = 128
WMAX = 32640          # gather window rows (<= int16 max, mult of 128)
DG = 24               # deferred-LN group size (tiles)


# ----------------------------------------------------------------------------
# host-side plan
# ----------------------------------------------------------------------------

def build_plan(src, dst, N):
    E = src.shape[0]
    n_blocks_real = (N + P - 1) // P
    n_blocks = ((n_blocks_real + NCORES - 1) // NCORES) * NCORES
    S = n_blocks // NCORES              # slots per core
    N_pad = n_blocks * P

    blk_of_edge = dst // P
    blk_counts = np.bincount(blk_of_edge, minlength=n_blocks)

    # balanced assignment: sort blocks by count desc, greedily fill cores
    order = np.argsort(-blk_counts, kind='stable')
    core_load = np.zeros(NCORES, dtype=np.int64)
    core_nblk = np.zeros(NCORES, dtype=np.int64)
    blk_core = np.zeros(n_blocks, dtype=np.int64)
    for b in order:
        cands = np.where(core_nblk < S)[0]
        c = cands[np.argmin(core_load[cands])]
        blk_core[b] = c
        core_load[c] += blk_counts[b]
        core_nblk[c] += 1

    # per-core slot order: blocks sorted by count desc
    slot_block = np.zeros((NCORES, S), dtype=np.int64)   # slot -> block id
    for c in range(NCORES):
        blks = np.where(blk_core == c)[0]
        blks = blks[np.argsort(-blk_counts[blks], kind='stable')]
        slot_block[c] = blks

    edge_core = blk_core[blk_of_edge]
    slot_of_block = np.zeros(n_blocks, dtype=np.int64)
    for c in range(NCORES):
        slot_of_block[slot_block[c]] = np.arange(S)
    edge_slot = slot_of_block[blk_of_edge]

    # compacted src table per core
    srclist = []
    for c in range(NCORES):
        u = np.unique(src[edge_core == c])
        srclist.append(u)
    E_TBL = ((max(len(u) for u in srclist) + P - 1) // P) * P
    n_win = max(1, (E_TBL + WMAX - 1) // WMAX)
    WSZ = ((E_TBL // n_win + P - 1) // P) * P
    assert WSZ <= 32767

    src_pos = np.zeros((NCORES, E), dtype=np.int64)
    for c in range(NCORES):
        m = edge_core == c
        src_pos[c, m] = np.searchsorted(srclist[c], src[m])

    cnt = np.zeros((NCORES, S, n_win), dtype=np.int64)
    for c in range(NCORES):
        m = edge_core == c
        w = src_pos[c, m] // WSZ
        np.add.at(cnt[c], (edge_slot[m], w), 1)
    tiles_sw = np.maximum(np.ceil(cnt / P).astype(np.int64).max(axis=0), 0)
    tiles_sw[:, 0] = np.maximum(tiles_sw[:, 0], 1)   # every slot >=1 tile
    TT = int(tiles_sw.sum())
    E_pad = TT * P

    sched = []
    off = 0
    TMAXG = 5
    for w in range(n_win):
        for s in range(S):
            t = int(tiles_sw[s, w])
            base = 0
            while t > 0:
                tc_ = min(t, TMAXG)
                # real edges of this call across cores
                mx = 0
                for c in range(NCORES):
                    rc = min(max(cnt[c, s, w] - base * P, 0), tc_ * P)
                    mx = max(mx, int(rc))
                ni = min(((mx + 15) // 16) * 16, tc_ * P)
                ni = max(ni, (tc_ - 1) * P + 16)
                sched.append((s, w, tc_, off, ni))
                off += tc_
                t -= tc_
                base += tc_

    canon_edge = np.full((NCORES, E_pad), -1, dtype=np.int64)
    for c in range(NCORES):
        m = np.where(edge_core == c)[0]
        w = src_pos[c, m] // WSZ
        key = w * S + edge_slot[m]
        ordr = np.argsort(key, kind='stable')
        me, ke = m[ordr], key[ordr]
        group_off = {}
        for (s_, w_, t_, o_, ni_) in sched:
            if (s_, w_) not in group_off:
                group_off[(s_, w_)] = o_ * P
        pos = np.zeros(len(me), dtype=np.int64)
        start = 0
        for k in np.unique(ke):
            cnt_k = int((ke == k).sum())
            w_, s_ = divmod(int(k), S)
            base = group_off[(s_, w_)]
            pos[start:start + cnt_k] = base + np.arange(cnt_k)
            start += cnt_k
        canon_edge[c, pos] = me
    return dict(
        N_pad=N_pad, n_blocks=n_blocks, S=S, E_TBL=E_TBL, n_win=n_win,
        WSZ=WSZ, TT=TT, E_pad=E_pad, sched=sched, slot_block=slot_block,
        srclist=srclist, src_pos=src_pos, canon_edge=canon_edge,
    )


def build_inputs(plan, inputs):
    node_feats = np.asarray(inputs['node_feats'], np.float32)
    edge_feats = np.asarray(inputs['edge_feats'], np.float32)
    src = np.asarray(inputs['src'])
    dst = np.asarray(inputs['dst'])
    N = node_feats.shape[0]

    tp = (np.asarray(inputs['time_feats'], np.float32) @
          np.asarray(inputs['W_tp'], np.float32) +
          np.asarray(inputs['b_tp'], np.float32))[0]
    bias_src = np.asarray(inputs['b_sg'], np.float32) + tp + \
        np.asarray(inputs['b_eg'], np.float32)

    W1b = np.concatenate([
        np.concatenate([inputs['W_sg'], inputs['W_du']], axis=1),
        np.concatenate([bias_src, inputs['b_du']])[None, :],
    ], axis=0).astype(np.float32)                      # [97, 192]
    W2b = np.concatenate([
        np.concatenate([inputs['W_dg'], inputs['W_su']], axis=1),
        np.concatenate([inputs['b_dg'], inputs['b_su']])[None, :],
    ], axis=0).astype(np.float32)                      # [97, 192]

    S, E_TBL, E_pad, TT = plan['S'], plan['E_TBL'], plan['E_pad'], plan['TT']
    nf_pad = np.zeros((plan['N_pad'], H), np.float32)
    nf_pad[:N] = node_feats

    iota = np.tile(np.arange(P, dtype=np.float32), (P, 1))
    ident = np.eye(P, dtype=np.float32)

    in_maps = []
    for c in range(NCORES):
        u = plan['srclist'][c]
        nftc = np.zeros((97, E_TBL), np.float32)
        nftc[:H, :len(u)] = node_feats[u].T
        nftc[96, :] = 1.0

        blocks = plan['slot_block'][c]
        own = nf_pad.reshape(-1, P, H)[blocks]          # [S, 128, 96]
        own_flat = own.reshape(S * P, H)
        nfbT = np.zeros((97, S * P), np.float32)
        nfbT[:H] = own_flat.T
        nfbT[96] = 1.0

        canon = plan['canon_edge'][c]
        real = canon >= 0
        ef_can = np.zeros((E_pad, H), np.float32)
        ef_can[real] = edge_feats[canon[real]]
        ef_pm = ef_can.reshape(TT, P, H).transpose(1, 0, 2).reshape(P, TT * H)
        nfb_pm = own.transpose(1, 0, 2).reshape(P, S * H)

        dstloc = np.full(E_pad, -1.0, np.float32)
        dstloc[real] = (dst[canon[real]] % P).astype(np.float32)
        dstloc = dstloc.reshape(TT, P).T.copy()         # [128, TT]

        gpos = np.zeros(E_pad, np.int64)
        gpos[real] = plan['src_pos'][c, canon[real]] % plan['WSZ']
        gidx = np.zeros((16, E_pad // 16), np.int16)
        idx_lin = np.arange(E_pad)
        gidx[idx_lin % 16, idx_lin // 16] = gpos.astype(np.int16)
        gidx = np.tile(gidx, (8, 1))                    # [128, E_pad/16]

        in_maps.append({
            'nftc': nftc.astype(BF16),
            'nfbT': nfbT.astype(BF16),
            'w1b': W1b.astype(BF16), 'w2b': W2b.astype(BF16),
            'weg': np.asarray(inputs['W_eg'], np.float32).astype(BF16),
            'efT': ef_can.T.astype(BF16).copy(),
            'ef_pm': ef_pm.astype(BF16),
            'dstloc': dstloc,
            'gidx': gidx,
            'iota': iota,
            'ident': ident.astype(BF16),
            'nfb': nfb_pm,
        })
    return in_maps


# ----------------------------------------------------------------------------
# device kernel
# ----------------------------------------------------------------------------

def build_kernel(plan):
    import concourse.bacc as bacc
    import concourse.bass as bass
    import concourse.mybir as mybir
    import concourse.tile as tile

    f32, bf16, i16 = mybir.dt.float32, mybir.dt.bfloat16, mybir.dt.int16
    AF = mybir.ActivationFunctionType
    ALU = mybir.AluOpType

    S, E_TBL, E_pad, TT = plan['S'], plan['E_TBL'], plan['E_pad'], plan['TT']
    n_win, WSZ = plan['n_win'], plan['WSZ']
    sched = plan['sched']
    NB = S * P

    nc = bacc.Bacc()
    dp = nc.declare_dram_parameter
    nftc = dp('nftc', [97, E_TBL], bf16, isOutput=False)
    nfbT = dp('nfbT', [97, NB], bf16, isOutput=False)
    w1b = dp('w1b', [97, 192], bf16, isOutput=False)
    w2b = dp('w2b', [97, 192], bf16, isOutput=False)
    weg = dp('weg', [H, H], bf16, isOutput=False)
    efT = dp('efT', [H, E_pad], bf16, isOutput=False)
    ef_pm = dp('ef_pm', [P, TT * H], bf16, isOutput=False)
    dstloc = dp('dstloc', [P, TT], f32, isOutput=False)
    gidx = dp('gidx', [P, E_pad // 16], i16, isOutput=False)
    iota = dp('iota', [P, P], f32, isOutput=False)
    ident = dp('ident', [P, P], bf16, isOutput=False)
    nfb = dp('nfb', [P, S * H], f32, isOutput=False)
    y_pm = dp('y_pm', [P, TT * H], bf16, isOutput=True)
    xout = dp('xout', [P, S * H], f32, isOutput=True)

    t1cw = []
    for w in range(n_win):
        wr = min(WSZ, E_TBL - w * WSZ)
        t1cw.append(nc.dram_tensor(f't1c{w}', [wr, 256], bf16))
    t2x = nc.dram_tensor('t2x', [P, S * 192], bf16)

    with tile.TileContext(nc) as tc:
        with (
            tc.tile_pool(name='const', bufs=1) as cpool,
            tc.tile_pool(name='io', bufs=2) as iop,
            tc.tile_pool(name='pa', bufs=2) as pa,
            tc.tile_pool(name='eft', bufs=2) as efp,
            tc.tile_pool(name='msb', bufs=7) as msp,
            tc.tile_pool(name='work', bufs=3) as wk,
            tc.tile_pool(name='grp', bufs=2) as grp,
            tc.tile_pool(name='yb', bufs=2) as ybp,
            tc.tile_pool(name='ps', bufs=3, space='PSUM') as pp,
            tc.tile_pool(name='pst', bufs=2, space='PSUM') as ppt,
            tc.tile_pool(name='psa', bufs=1, space='PSUM') as ppa,
            tc.tile_pool(name='ps_sum', bufs=2, space='PSUM') as pps,
        ):
            # ---- constants ----
            iota_sb = cpool.tile([P, P], f32, tag='iota')
            nc.sync.dma_start(out=iota_sb[:], in_=iota[:])
            id_bf = cpool.tile([P, P], bf16, tag='idb')
            nc.sync.dma_start(out=id_bf[:], in_=ident[:])
            w1_sb = cpool.tile([97, 192], bf16, tag='w1')
            nc.sync.dma_start(out=w1_sb[:], in_=w1b[:])
            w2_sb = cpool.tile([97, 192], bf16, tag='w2')
            nc.sync.dma_start(out=w2_sb[:], in_=w2b[:])
            weg_sb = cpool.tile([H, H], bf16, tag='weg')
            nc.sync.dma_start(out=weg_sb[:], in_=weg[:])
            idx_all = cpool.tile([P, E_pad // 16], i16, tag='gidx')
            nc.sync.dma_start(out=idx_all[:], in_=gidx[:])
            dl_all = cpool.tile([P, TT], f32, tag='dstloc')
            nc.sync.dma_start(out=dl_all[:], in_=dstloc[:])
            eps_col = cpool.tile([P, 1], f32, tag='eps')
            nc.vector.memset(eps_col[:], LN_EPS)
            eps6_col = cpool.tile([P, 1], f32, tag='eps6')
            nc.vector.memset(eps6_col[:], 1e-6)
            # acc[s]: bf16 partial sums per slot (windows < last); after
            # finalize the first 96 cols hold xpre for the final phase
            acc = cpool.tile([P, S * 192], bf16, tag='acc')

            # ---- phase A: node transform tables ----
            ACH = 16
            phase_a = []
            for w in range(n_win):
                wr = min(WSZ, E_TBL - w * WSZ)
                phase_a.append(
                    ('t1', nftc, w1_sb, w * WSZ // P, wr // P, 256, w))
            phase_a.insert(1, ('t2', nfbT, w2_sb, 0, S, 192, None))
            naring, tbring = [], []
            for r in range(3):
                nt = cpool.tile([97, ACH * P], bf16, tag=f'nfa{r}')
                naring.append(nt)
                tb = cpool.tile([P, ACH * 256], bf16, tag=f'tb{r}')
                nc.vector.memset(tb[:], 0)
                tbring.append(tb)
            ring_j = [0]
            for (mode, srcT, wsb, tile0, n_tiles, dcols, wid) in phase_a:
                for j0 in range(0, n_tiles, ACH):
                    jn = min(ACH, n_tiles - j0)
                    nchunk = naring[ring_j[0] % 3]
                    tbuf = tbring[ring_j[0] % 3]
                    ring_j[0] += 1
                    nc.scalar.dma_start(
                        out=nchunk[:, :jn * P],
                        in_=srcT[:, (tile0 + j0) * P:(tile0 + j0 + jn) * P])
                    for k in range(0, jn, 2):
                        kn = min(2, jn - k)
                        mm = ppa.tile([P, 2 * 192], f32, space='PSUM',
                                      tag='pamm')
                        for q in range(kn):
                            nc.tensor.matmul(
                                out=mm[:, q * 192:(q + 1) * 192],
                                lhsT=nchunk[:, (k + q) * P:(k + q + 1) * P],
                                rhs=wsb[:], start=True, stop=True)
                        nc.vector.tensor_copy(
                            out=tbuf[:, k * dcols:k * dcols + kn * dcols]
                            .rearrange('p (j d) -> p j d', d=dcols)[:, :, 0:192]
                            if dcols == 256 else
                            tbuf[:, k * dcols:(k + kn) * dcols],
                            in_=mm[:, :kn * 192].rearrange(
                                'p (j d) -> p j d', d=192)
                            if dcols == 256 else mm[:, :kn * 192])
                    if mode == 't1':
                        nc.sync.dma_start(
                            out=t1cw[wid][j0 * P:(j0 + jn) * P, :].rearrange(
                                '(j p) d -> p j d', p=P),
                            in_=tbuf[:, :jn * 256].rearrange(
                                'p (j d) -> p j d', d=256))
                    else:
                        nc.sync.dma_start(
                            out=t2x[:, j0 * 192:(j0 + jn) * 192],
                            in_=tbuf[:, :jn * 192])

            # ---- phase B ----
            sw_last = {}
            sw_haveprev = {}
            for (s, w, t, off, ni) in sched:
                sw_last[(s, w)] = off + t - 1
                sw_haveprev[s] = {}
            last_w = {}
            for (s, w) in sw_last:
                last_w[s] = max(last_w.get(s, 0), w)
            seen_w = {}
            for (s, w) in sorted(sw_last):
                sw_haveprev[(s, w)] = any(
                    (s, w2) in sw_last for w2 in range(w))
            # DIY gather ring (stale-safe: memset once)
            TMAXG = 5
            gring = []
            for r in range(6):
                gt = cpool.tile([P, TMAXG * 256], bf16, tag=f'gring{r}')
                nc.vector.memset(gt[:], 0)
                gring.append(gt)
            gring_i = [0]

            pending = []     # (off, t, msb, efg) per (s,w) group
            pend_n = [0]
            stats_buf = [None]

            def ln_coeffs(st, g):
                """Batched LN: stats [P, g, 6] -> (rstd, nmr) [P, g]."""
                stv = st[:].rearrange('p (g s) -> p g s', s=6)
                a1 = grp.tile([P, DG], f32, tag='a1')
                nc.vector.tensor_add(
                    out=a1[:, :g], in0=stv[:, :g, 2], in1=stv[:, :g, 5])
                a2 = grp.tile([P, DG], f32, tag='a2')
                nc.vector.tensor_sub(
                    out=a2[:, :g], in0=stv[:, :g, 1], in1=stv[:, :g, 4])
                a3 = grp.tile([P, DG], f32, tag='a3')
                nc.vector.tensor_mul(
                    out=a3[:, :g], in0=a2[:, :g], in1=a2[:, :g])
                var = grp.tile([P, DG], f32, tag='var')
                nc.vector.tensor_scalar(
                    out=var[:, :g], in0=a1[:, :g], scalar1=1.0 / 96.0,
                    scalar2=None, op0=ALU.mult)
                nc.vector.tensor_scalar(
                    out=a3[:, :g], in0=a3[:, :g], scalar1=0.25,
                    scalar2=None, op0=ALU.mult)
                nc.vector.tensor_add(
                    out=var[:, :g], in0=var[:, :g], in1=a3[:, :g])
                std = grp.tile([P, DG], f32, tag='std')
                nc.scalar.activation(
                    out=std[:, :g], in_=var[:, :g], func=AF.Sqrt,
                    bias=eps_col[:])
                rstd = grp.tile([P, DG], f32, tag='rstd')
                nc.vector.reciprocal(out=rstd[:, :g], in_=std[:, :g])
                msum = grp.tile([P, DG], f32, tag='msum')
                nc.vector.tensor_add(
                    out=msum[:, :g], in0=stv[:, :g, 1], in1=stv[:, :g, 4])
                nmr = grp.tile([P, DG], f32, tag='nmr')
                nc.vector.tensor_mul(
                    out=nmr[:, :g], in0=msum[:, :g], in1=rstd[:, :g])
                nc.vector.tensor_scalar(
                    out=nmr[:, :g], in0=nmr[:, :g], scalar1=-0.5,
                    scalar2=None, op0=ALU.mult)
                return rstd, nmr

            def flush():
                if not pending:
                    return
                g = pend_n[0]
                rstd, nmr = ln_coeffs(stats_buf[0], g)
                ybuf = ybp.tile([P, DG * H], bf16, tag='ybuf')
                j = 0
                off0 = pending[0][0]
                for (off_, t_, msb_, efg_) in pending:
                    for k in range(t_):
                        nc.scalar.activation(
                            out=ybuf[:, (j + k) * H:(j + k + 1) * H],
                            in_=msb_[:, k * H:(k + 1) * H],
                            func=AF.Silu,
                            bias=nmr[:, j + k:j + k + 1],
                            scale=rstd[:, j + k:j + k + 1])
                    nc.vector.tensor_add(
                        out=ybuf[:, j * H:(j + t_) * H],
                        in0=ybuf[:, j * H:(j + t_) * H],
                        in1=efg_)
                    j += t_
                nc.sync.dma_start(
                    out=y_pm[:, off0 * H:(off0 + g) * H],
                    in_=ybuf[:, :g * H])
                pending.clear()
                pend_n[0] = 0
                stats_buf[0] = None

            cur_key = None
            cur_span = None
            cur_s8 = [-1]
            t2base = 0
            sums = None
            sw_start = [False]
            win_base = [None]
            win_len = [0]
            win_eftg = [None]
            win_efg = [None]
            for (s, w, t, off, ni) in sched:
                if pend_n[0] + t > DG:
                    flush()
                if (s, w) != cur_key:
                    if cur_span is None or s // 8 != cur_s8[0]:
                        t2span = iop.tile([P, 8 * 192], bf16, tag='t2span')
                        s8 = (s // 8) * 8
                        cur_s8[0] = s // 8
                        sn = min(8, S - s8)
                        nc.sync.dma_start(
                            out=t2span[:, :sn * 192],
                            in_=t2x[:, s8 * 192:(s8 + sn) * 192])
                        cur_span = t2span
                    cur_key = (s, w)
                    t2base = (s % 8) * 192
                    sums = pps.tile([P, 192], f32, space='PSUM', tag='sums')
                    sw_start[0] = True
                gbuf = gring[gring_i[0] % 6]
                gring_i[0] += 1
                nc.gpsimd.dma_gather(
                    out_ap=gbuf[:, :t * 256].rearrange(
                        'p (t d) -> p t d', t=t),
                    in_ap=t1cw[w][:],
                    idxs_ap=idx_all[:, off * 8:off * 8 + (ni + 15) // 16],
                    num_idxs=ni,
                    num_idxs_reg=ni,
                    elem_size=256,
                    single_packet=(ni <= 512),
                )
                if win_base[0] is None or off >= win_base[0] + win_len[0]:
                    wb = off
                    wl = 0
                    for (s2, w2, t2, off2, ni2) in sched:
                        if off2 < wb:
                            continue
                        if wl + t2 > DG:
                            break
                        wl += t2
                    win_base[0] = wb
                    win_len[0] = wl
                    eftg_w = efp.tile([H, DG * P], bf16, tag='eftgw')
                    nc.sync.dma_start(
                        out=eftg_w[:, :wl * P],
                        in_=efT[:, wb * P:(wb + wl) * P])
                    efg_w = efp.tile([P, DG * H], bf16, tag='efgw')
                    nc.sync.dma_start(
                        out=efg_w[:, :wl * H],
                        in_=ef_pm[:, wb * H:(wb + wl) * H])
                    win_eftg[0] = eftg_w
                    win_efg[0] = efg_w
                lo = off - win_base[0]

                # batched one-hot for the group
                onehot = wk.tile([P, t * P], bf16, tag='onehot')
                nc.vector.tensor_tensor(
                    out=onehot[:].rearrange('p (t q) -> p t q', q=P),
                    in0=dl_all[:, off:off + t, None].to_broadcast([P, t, P]),
                    in1=iota_sb[:, None, :].to_broadcast([P, t, P]),
                    op=ALU.is_equal)
                trps = ppt.tile([P, t * P], bf16, space='PSUM', tag='tr')
                for k in range(t):
                    nc.tensor.transpose(
                        out=trps[:, k * P:(k + 1) * P],
                        in_=onehot[:, k * P:(k + 1) * P],
                        identity=id_bf[:])
                ohne = wk.tile([P, t * P], bf16, tag='ohne')
                nc.vector.tensor_copy(out=ohne[:], in_=trps[:])

                mp = pp.tile([P, t * H], f32, space='PSUM', tag='mm')
                for k in range(t):
                    nc.tensor.matmul(
                        out=mp[:, k * H:(k + 1) * H],
                        lhsT=win_eftg[0][:, (lo + k) * P:(lo + k + 1) * P],
                        rhs=weg_sb[:], start=True, stop=False)
                    nc.tensor.matmul(
                        out=mp[:, k * H:(k + 1) * H],
                        lhsT=ohne[:, k * P:(k + 1) * P],
                        rhs=cur_span[:, t2base:t2base + H],
                        start=False, stop=True)
                msb = msp.tile([P, t * H], f32, tag='msb')
                nc.vector.tensor_add(
                    out=msb[:].rearrange('p (t f) -> p t f', f=H),
                    in0=mp[:].rearrange('p (t f) -> p t f', f=H),
                    in1=gbuf[:, :t * 256].rearrange('p (t d) -> p t d', d=256)[:, :, 0:H])

                valcat = wk.tile([P, t * 192], bf16, tag='valcat')
                vv = valcat[:].rearrange('p (t d) -> p t d', d=192)
                nc.scalar.activation(
                    out=vv[:, :, 0:H],
                    in_=msb[:].rearrange('p (t f) -> p t f', f=H),
                    func=AF.Sigmoid)
                nc.vector.tensor_tensor(
                    out=vv[:, :, H:192],
                    in0=gbuf[:, :t * 256].rearrange('p (t d) -> p t d', d=256)[:, :, H:192],
                    in1=vv[:, :, 0:H], op=ALU.mult)

                for k in range(t):
                    tt = off + k
                    nc.tensor.matmul(
                        out=sums[:],
                        lhsT=onehot[:, k * P:(k + 1) * P],
                        rhs=valcat[:, k * 192:(k + 1) * 192],
                        start=sw_start[0],
                        stop=(tt == sw_last[(s, w)]))
                    sw_start[0] = False

                if stats_buf[0] is None:
                    st_new = grp.tile([P, DG * 6], f32, tag='stats')
                    stats_buf[0] = st_new
                j = pend_n[0]
                for k0 in range(t):
                    nc.vector.bn_stats(
                        out=stats_buf[0][:, (j + k0) * 6:(j + k0 + 1) * 6],
                        in_=msb[:, k0 * H:(k0 + 1) * H])
                pending.append((off, t, msb,
                                win_efg[0][:, lo * H:(lo + t) * H]))
                pend_n[0] = j + t

                if off + t - 1 == sw_last[(s, w)]:
                    if w < last_w[s]:
                        # stash partial sums (bf16) for later windows
                        nc.vector.tensor_copy(
                            out=acc[:, s * 192:(s + 1) * 192], in_=sums[:])
                    else:
                        if sw_haveprev[(s, w)]:
                            tot = wk.tile([P, 192], f32, tag='tot')
                            nc.vector.tensor_add(
                                out=tot[:], in0=sums[:],
                                in1=acc[:, s * 192:(s + 1) * 192])
                            ss_ap, ssh_ap = tot[:, 0:H], tot[:, H:192]
                        else:
                            ss_ap, ssh_ap = sums[:, 0:H], sums[:, H:192]
                        ssd = wk.tile([P, H], f32, tag='ssd')
                        nc.scalar.activation(
                            out=ssd[:], in_=ss_ap, func=AF.Identity,
                            bias=eps6_col[:])
                        rec = wk.tile([P, H], f32, tag='rec')
                        nc.vector.reciprocal(out=rec[:], in_=ssd[:])
                        h = wk.tile([P, H], f32, tag='h')
                        nc.vector.tensor_mul(
                            out=h[:], in0=ssh_ap, in1=rec[:])
                        nc.vector.tensor_add(
                            out=acc[:, s * 192:s * 192 + H],
                            in0=h[:], in1=cur_span[:, t2base + H:t2base + 192])
            flush()

            # ---- final phase: node LN + silu + residual ----
            FG = 16
            for s0 in range(0, S, FG):
                g = min(FG, S - s0)
                st = grp.tile([P, DG * 6], f32, tag='stats')
                for k0 in range(g):
                    nc.vector.bn_stats(
                        out=st[:, k0 * 6:(k0 + 1) * 6],
                        in_=acc[:, (s0 + k0) * 192:(s0 + k0) * 192 + H])
                rstd, nmr = ln_coeffs(st, g)
                nfblk = ybp.tile([P, FG * H], f32, tag='nfblk')
                nc.sync.dma_start(
                    out=nfblk[:, :g * H],
                    in_=nfb[:, s0 * H:(s0 + g) * H])
                xbuf = ybp.tile([P, FG * H], f32, tag='xbuf')
                for k in range(g):
                    s = s0 + k
                    nc.scalar.activation(
                        out=xbuf[:, k * H:(k + 1) * H],
                        in_=acc[:, s * 192:s * 192 + H],
                        func=AF.Silu, bias=nmr[:, k:k + 1],
                        scale=rstd[:, k:k + 1])
                nc.vector.tensor_add(
                    out=xbuf[:, :g * H], in0=xbuf[:, :g * H],
                    in1=nfblk[:, :g * H])
                nc.sync.dma_start(
                    out=xout[:, s0 * H:(s0 + g) * H],
                    in_=xbuf[:, :g * H])

    nc.finalize()
    return nc


# ----------------------------------------------------------------------------
# top-level
# ----------------------------------------------------------------------------

_TRACE = [False]


def kernel(**inputs):
    from concourse.bass_utils import run_bass_kernel_spmd

    src = np.asarray(inputs['src'])
    dst = np.asarray(inputs['dst'])
    node_feats = np.asarray(inputs['node_feats'], np.float32)
    edge_feats = np.asarray(inputs['edge_feats'], np.float32)
    N, E = node_feats.shape[0], edge_feats.shape[0]

    plan = build_plan(src, dst, N)
    in_maps = build_inputs(plan, inputs)
    nc = build_kernel(plan)
    res = run_bass_kernel_spmd(
        nc, in_maps, core_ids=list(range(NCORES)), trace=_TRACE[0])
    kernel.last_result = res

    x = np.zeros((N, H), np.float32)
    y = np.zeros((E, H), np.float32)
    for c in range(NCORES):
        out = res.results[c]
        blocks = plan['slot_block'][c]
        xs = out['xout'].reshape(P, plan['S'], H).transpose(1, 0, 2)
        for s_i, b in enumerate(blocks):
            lo = b * P
            hi = min(lo + P, N)
            if lo < N:
                x[lo:hi] = xs[s_i, :hi - lo]
        canon = plan['canon_edge'][c]
        real = canon >= 0
        y_can = np.asarray(out['y_pm']).reshape(
            P, plan['TT'], H).transpose(1, 0, 2).reshape(plan['E_pad'], H)
        y[canon[real]] = y_can[real].astype(np.float32)
    return x, y


# revision 39
# speedup vs baseline: 1.0942x; 1.0942x over previous
"""ALIGNN edge-gated message passing on 8 Trainium2 NeuronCores.

Strategy: edges partitioned by dst-block across cores (no collectives).
Each core receives host-prepared, per-core data:
  - a compacted node table (only nodes referenced as src by its edges)
  - its own 98 node-blocks (128 nodes each) in load-balanced "slot" order
  - its edge slice in canonical (slot, window, tile) order
Device work per core:
  phase A : PE matmuls node_feats -> T1=[e_src+bias | Bh] (f32, compact rows)
            and T2x=[e_dst | x_lin] (bf16, own blocks)
  phase B : per (slot,window) group of 128-edge tiles: dma_gather T1 rows by
            src (int16 windows), one-hot(dst_local) via batched is_equal,
            m = ef@W_eg + onehot.T@T2blk (+ gathered), sigma = sigmoid(m),
            segment-sum via one-hot matmul accumulation in PSUM;
            LN sqrt batched over deferred groups of 32 tiles, then
            y = ef + silu(LN(m)) written edge-major
  phase D : per slot: xpre = x_lin + ssh/(ss+1e-6); final phase applies
            LN + silu + residual for all slots with batched stats
"""
import sys

if '/opt/trn_rl_repo' not in sys.path:
    sys.path.insert(0, '/opt/trn_rl_repo')

import numpy as np
import ml_dtypes

BF16 = ml_dtypes.bfloat16
H = 96
LN_EPS = 1e-5
NCORES = 8
P = 128
WMAX = 32640          # gather window rows (<= int16 max, mult of 128)
DG = 24               # deferred-LN group size (tiles)


# ----------------------------------------------------------------------------
# host-side plan
# ----------------------------------------------------------------------------

def build_plan(src, dst, N):
    E = src.shape[0]
    n_blocks_real = (N + P - 1) // P
    n_blocks = ((n_blocks_real + NCORES - 1) // NCORES) * NCORES
    S = n_blocks // NCORES              # slots per core
    N_pad = n_blocks * P

    blk_of_edge = dst // P
    blk_counts = np.bincount(blk_of_edge, minlength=n_blocks)

    # balanced assignment: sort blocks by count desc, greedily fill cores
    order = np.argsort(-blk_counts, kind='stable')
    core_load = np.zeros(NCORES, dtype=np.int64)
    core_nblk = np.zeros(NCORES, dtype=np.int64)
    blk_core = np.zeros(n_blocks, dtype=np.int64)
    for b in order:
        cands = np.where(core_nblk < S)[0]
        c = cands[np.argmin(core_load[cands])]
        blk_core[b] = c
        core_load[c] += blk_counts[b]
        core_nblk[c] += 1

    # per-core slot order: blocks sorted by count desc
    slot_block = np.zeros((NCORES, S), dtype=np.int64)   # slot -> block id
    for c in range(NCORES):
        blks = np.where(blk_core == c)[0]
        blks = blks[np.argsort(-blk_counts[blks], kind='stable')]
        slot_block[c] = blks

    edge_core = blk_core[blk_of_edge]
    slot_of_block = np.zeros(n_blocks, dtype=np.int64)
    for c in range(NCORES):
        slot_of_block[slot_block[c]] = np.arange(S)
    edge_slot = slot_of_block[blk_of_edge]

    # compacted src table per core
    srclist = []
    for c in range(NCORES):
        u = np.unique(src[edge_core == c])
        srclist.append(u)
    E_TBL = ((max(len(u) for u in srclist) + P - 1) // P) * P
    n_win = max(1, (E_TBL + WMAX - 1) // WMAX)
    WSZ = ((E_TBL // n_win + P - 1) // P) * P
    assert WSZ <= 32767

    src_pos = np.zeros((NCORES, E), dtype=np.int64)
    for c in range(NCORES):
        m = edge_core == c
        src_pos[c, m] = np.searchsorted(srclist[c], src[m])

    cnt = np.zeros((NCORES, S, n_win), dtype=np.int64)
    for c in range(NCORES):
        m = edge_core == c
        w = src_pos[c, m] // WSZ
        np.add.at(cnt[c], (edge_slot[m], w), 1)
    tiles_sw = np.maximum(np.ceil(cnt / P).astype(np.int64).max(axis=0), 0)
    tiles_sw[:, 0] = np.maximum(tiles_sw[:, 0], 1)   # every slot >=1 tile
    TT = int(tiles_sw.sum())
    E_pad = TT * P

    sched = []
    off = 0
    TMAXG = 5
    for w in range(n_win):
        for s in range(S):
            t = int(tiles_sw[s, w])
            base = 0
            while t > 0:
                tc_ = min(t, TMAXG)
                # real edges of this call across cores
                mx = 0
                for c in range(NCORES):
                    rc = min(max(cnt[c, s, w] - base * P, 0), tc_ * P)
                    mx = max(mx, int(rc))
                ni = min(((mx + 15) // 16) * 16, tc_ * P)
                ni = max(ni, (tc_ - 1) * P + 16)
                sched.append((s, w, tc_, off, ni))
                off += tc_
                t -= tc_
                base += tc_

    canon_edge = np.full((NCORES, E_pad), -1, dtype=np.int64)
    for c in range(NCORES):
        m = np.where(edge_core == c)[0]
        w = src_pos[c, m] // WSZ
        key = w * S + edge_slot[m]
        ordr = np.argsort(key, kind='stable')
        me, ke = m[ordr], key[ordr]
        group_off = {}
        for (s_, w_, t_, o_, ni_) in sched:
            if (s_, w_) not in group_off:
                group_off[(s_, w_)] = o_ * P
        pos = np.zeros(len(me), dtype=np.int64)
        start = 0
        for k in np.unique(ke):
            cnt_k = int((ke == k).sum())
            w_, s_ = divmod(int(k), S)
            base = group_off[(s_, w_)]
            pos[start:start + cnt_k] = base + np.arange(cnt_k)
            start += cnt_k
        canon_edge[c, pos] = me
    return dict(
        N_pad=N_pad, n_blocks=n_blocks, S=S, E_TBL=E_TBL, n_win=n_win,
        WSZ=WSZ, TT=TT, E_pad=E_pad, sched=sched, slot_block=slot_block,
        srclist=srclist, src_pos=src_pos, canon_edge=canon_edge,
    )


def build_inputs(plan, inputs):
    node_feats = np.asarray(inputs['node_feats'], np.float32)
    edge_feats = np.asarray(inputs['edge_feats'], np.float32)
    src = np.asarray(inputs['src'])
    dst = np.asarray(inputs['dst'])
    N = node_feats.shape[0]

    tp = (np.asarray(inputs['time_feats'], np.float32) @
          np.asarray(inputs['W_tp'], np.float32) +
          np.asarray(inputs['b_tp'], np.float32))[0]
    bias_src = np.asarray(inputs['b_sg'], np.float32) + tp + \
        np.asarray(inputs['b_eg'], np.float32)

    W1b = np.concatenate([
        np.concatenate([inputs['W_sg'], inputs['W_du']], axis=1),
        np.concatenate([bias_src, inputs['b_du']])[None, :],
    ], axis=0).astype(np.float32)                      # [97, 192]
    W2b = np.concatenate([
        np.concatenate([inputs['W_dg'], inputs['W_su']], axis=1),
        np.concatenate([inputs['b_dg'], inputs['b_su']])[None, :],
    ], axis=0).astype(np.float32)                      # [97, 192]

    S, E_TBL, E_pad, TT = plan['S'], plan['E_TBL'], plan['E_pad'], plan['TT']
    nf_pad = np.zeros((plan['N_pad'], H), np.float32)
    nf_pad[:N] = node_feats

    iota = np.tile(np.arange(P, dtype=np.float32), (P, 1))
    ident = np.eye(P, dtype=np.float32)

    in_maps = []
    for c in range(NCORES):
        u = plan['srclist'][c]
        nftc = np.zeros((97, E_TBL), np.float32)
        nftc[:H, :len(u)] = node_feats[u].T
        nftc[96, :] = 1.0

        blocks = plan['slot_block'][c]
        own = nf_pad.reshape(-1, P, H)[blocks]          # [S, 128, 96]
        own_flat = own.reshape(S * P, H)
        nfbT = np.zeros((97, S * P), np.float32)
        nfbT[:H] = own_flat.T
        nfbT[96] = 1.0

        canon = plan['canon_edge'][c]
        real = canon >= 0
        ef_can = np.zeros((E_pad, H), np.float32)
        ef_can[real] = edge_feats[canon[real]]
        ef_pm = ef_can.reshape(TT, P, H).transpose(1, 0, 2).reshape(P, TT * H)
        nfb_pm = own.transpose(1, 0, 2).reshape(P, S * H)

        dstloc = np.full(E_pad, -1.0, np.float32)
        dstloc[real] = (dst[canon[real]] % P).astype(np.float32)
        dstloc = dstloc.reshape(TT, P).T.copy()         # [128, TT]

        gpos = np.zeros(E_pad, np.int64)
        gpos[real] = plan['src_pos'][c, canon[real]] % plan['WSZ']
        gidx = np.zeros((16, E_pad // 16), np.int16)
        idx_lin = np.arange(E_pad)
        gidx[idx_lin % 16, idx_lin // 16] = gpos.astype(np.int16)
        gidx = np.tile(gidx, (8, 1))                    # [128, E_pad/16]

        in_maps.append({
            'nftc': nftc.astype(BF16),
            'nfbT': nfbT.astype(BF16),
            'w1b': W1b.astype(BF16), 'w2b': W2b.astype(BF16),
            'weg': np.asarray(inputs['W_eg'], np.float32).astype(BF16),
            'efT': ef_can.T.astype(BF16).copy(),
            'ef_pm': ef_pm.astype(BF16),
            'dstloc': dstloc,
            'gidx': gidx,
            'iota': iota,
            'ident': ident.astype(BF16),
            'nfb': nfb_pm,
        })
    return in_maps


# ----------------------------------------------------------------------------
# device kernel
# ----------------------------------------------------------------------------

def build_kernel(plan):
    import concourse.bacc as bacc
    import concourse.bass as bass
    import concourse.mybir as mybir
    import concourse.tile as tile

    f32, bf16, i16 = mybir.dt.float32, mybir.dt.bfloat16, mybir.dt.int16
    AF = mybir.ActivationFunctionType
    ALU = mybir.AluOpType

    S, E_TBL, E_pad, TT = plan['S'], plan['E_TBL'], plan['E_pad'], plan['TT']
    n_win, WSZ = plan['n_win'], plan['WSZ']
    sched = plan['sched']
    NB = S * P

    nc = bacc.Bacc()
    dp = nc.declare_dram_parameter
    nftc = dp('nftc', [97, E_TBL], bf16, isOutput=False)
    nfbT = dp('nfbT', [97, NB], bf16, isOutput=False)
    w1b = dp('w1b', [97, 192], bf16, isOutput=False)
    w2b = dp('w2b', [97, 192], bf16, isOutput=False)
    weg = dp('weg', [H, H], bf16, isOutput=False)
    efT = dp('efT', [H, E_pad], bf16, isOutput=False)
    ef_pm = dp('ef_pm', [P, TT * H], bf16, isOutput=False)
    dstloc = dp('dstloc', [P, TT], f32, isOutput=False)
    gidx = dp('gidx', [P, E_pad // 16], i16, isOutput=False)
    iota = dp('iota', [P, P], f32, isOutput=False)
    ident = dp('ident', [P, P], bf16, isOutput=False)
    nfb = dp('nfb', [P, S * H], f32, isOutput=False)
    y_pm = dp('y_pm', [P, TT * H], bf16, isOutput=True)
    xout = dp('xout', [P, S * H], f32, isOutput=True)

    t1cw = []
    for w in range(n_win):
        wr = min(WSZ, E_TBL - w * WSZ)
        t1cw.append(nc.dram_tensor(f't1c{w}', [wr, 256], bf16))
    t2x = nc.dram_tensor('t2x', [P, S * 192], bf16)

    with tile.TileContext(nc) as tc:
        with (
            tc.tile_pool(name='const', bufs=1) as cpool,
            tc.tile_pool(name='io', bufs=2) as iop,
            tc.tile_pool(name='pa', bufs=2) as pa,
            tc.tile_pool(name='eft', bufs=2) as efp,
            tc.tile_pool(name='msb', bufs=7) as msp,
            tc.tile_pool(name='work', bufs=3) as wk,
            tc.tile_pool(name='grp', bufs=2) as grp,
            tc.tile_pool(name='yb', bufs=2) as ybp,
            tc.tile_pool(name='ps', bufs=3, space='PSUM') as pp,
            tc.tile_pool(name='pst', bufs=2, space='PSUM') as ppt,
            tc.tile_pool(name='psa', bufs=1, space='PSUM') as ppa,
            tc.tile_pool(name='ps_sum', bufs=2, space='PSUM') as pps,
        ):
            # ---- constants ----
            iota_sb = cpool.tile([P, P], f32, tag='iota')
            nc.sync.dma_start(out=iota_sb[:], in_=iota[:])
            id_bf = cpool.tile([P, P], bf16, tag='idb')
            nc.sync.dma_start(out=id_bf[:], in_=ident[:])
            w1_sb = cpool.tile([97, 192], bf16, tag='w1')
            nc.sync.dma_start(out=w1_sb[:], in_=w1b[:])
            w2_sb = cpool.tile([97, 192], bf16, tag='w2')
            nc.sync.dma_start(out=w2_sb[:], in_=w2b[:])
            weg_sb = cpool.tile([H, H], bf16, tag='weg')
            nc.sync.dma_start(out=weg_sb[:], in_=weg[:])
            idx_all = cpool.tile([P, E_pad // 16], i16, tag='gidx')
            nc.sync.dma_start(out=idx_all[:], in_=gidx[:])
            dl_all = cpool.tile([P, TT], f32, tag='dstloc')
            nc.sync.dma_start(out=dl_all[:], in_=dstloc[:])
            eps_col = cpool.tile([P, 1], f32, tag='eps')
            nc.vector.memset(eps_col[:], LN_EPS)
            eps6_col = cpool.tile([P, 1], f32, tag='eps6')
            nc.vector.memset(eps6_col[:], 1e-6)
            # acc[s]: bf16 partial sums per slot (windows < last); after
            # finalize the first 96 cols hold xpre for the final phase
            acc = cpool.tile([P, S * 192], bf16, tag='acc')

            # ---- phase A: node transform tables ----
            ACH = 16
            phase_a = []
            for w in range(n_win):
                wr = min(WSZ, E_TBL - w * WSZ)
                phase_a.append(
                    ('t1', nftc, w1_sb, w * WSZ // P, wr // P, 256, w))
            phase_a.insert(1, ('t2', nfbT, w2_sb, 0, S, 192, None))
            naring, tbring = [], []
            for r in range(3):
                nt = cpool.tile([97, ACH * P], bf16, tag=f'nfa{r}')
                naring.append(nt)
                tb = cpool.tile([P, ACH * 256], bf16, tag=f'tb{r}')
                nc.vector.memset(tb[:], 0)
                tbring.append(tb)
            ring_j = [0]
            for (mode, srcT, wsb, tile0, n_tiles, dcols, wid) in phase_a:
                for j0 in range(0, n_tiles, ACH):
                    jn = min(ACH, n_tiles - j0)
                    nchunk = naring[ring_j[0] % 3]
                    tbuf = tbring[ring_j[0] % 3]
                    ring_j[0] += 1
                    nc.scalar.dma_start(
                        out=nchunk[:, :jn * P],
                        in_=srcT[:, (tile0 + j0) * P:(tile0 + j0 + jn) * P])
                    for k in range(0, jn, 2):
                        kn = min(2, jn - k)
                        mm = ppa.tile([P, 2 * 192], f32, space='PSUM',
                                      tag='pamm')
                        for q in range(kn):
                            nc.tensor.matmul(
                                out=mm[:, q * 192:(q + 1) * 192],
                                lhsT=nchunk[:, (k + q) * P:(k + q + 1) * P],
                                rhs=wsb[:], start=True, stop=True)
                        nc.vector.tensor_copy(
                            out=tbuf[:, k * dcols:k * dcols + kn * dcols]
                            .rearrange('p (j d) -> p j d', d=dcols)[:, :, 0:192]
                            if dcols == 256 else
                            tbuf[:, k * dcols:(k + kn) * dcols],
                            in_=mm[:, :kn * 192].rearrange(
                                'p (j d) -> p j d', d=192)
                            if dcols == 256 else mm[:, :kn * 192])
                    if mode == 't1':
                        nc.sync.dma_start(
                            out=t1cw[wid][j0 * P:(j0 + jn) * P, :].rearrange(
                                '(j p) d -> p j d', p=P),
                            in_=tbuf[:, :jn * 256].rearrange(
                                'p (j d) -> p j d', d=256))
                    else:
                        nc.sync.dma_start(
                            out=t2x[:, j0 * 192:(j0 + jn) * 192],
                            in_=tbuf[:, :jn * 192])

            # ---- phase B ----
            sw_last = {}
            sw_haveprev = {}
            for (s, w, t, off, ni) in sched:
                sw_last[(s, w)] = off + t - 1
                sw_haveprev[s] = {}
            last_w = {}
            for (s, w) in sw_last:
                last_w[s] = max(last_w.get(s, 0), w)
            seen_w = {}
            for (s, w) in sorted(sw_last):
                sw_haveprev[(s, w)] = any(
                    (s, w2) in sw_last for w2 in range(w))
            # DIY gather ring (stale-safe: memset once)
            TMAXG = 5
            gring = []
            for r in range(5):
                gt = cpool.tile([P, TMAXG * 256], bf16, tag=f'gring{r}')
                nc.vector.memset(gt[:], 0)
                gring.append(gt)
            gring_i = [0]

            pending = []     # (off, t, msb, efg) per (s,w) group
            pend_n = [0]
            stats_buf = [None]

            def ln_coeffs(st, g):
                """Batched LN: stats [P, g, 6] -> (rstd, nmr) [P, g]."""
                stv = st[:].rearrange('p (g s) -> p g s', s=6)
                a1 = grp.tile([P, DG], f32, tag='a1')
                nc.vector.tensor_add(
                    out=a1[:, :g], in0=stv[:, :g, 2], in1=stv[:, :g, 5])
                a2 = grp.tile([P, DG], f32, tag='a2')
                nc.vector.tensor_sub(
                    out=a2[:, :g], in0=stv[:, :g, 1], in1=stv[:, :g, 4])
                a3 = grp.tile([P, DG], f32, tag='a3')
                nc.vector.tensor_mul(
                    out=a3[:, :g], in0=a2[:, :g], in1=a2[:, :g])
                var = grp.tile([P, DG], f32, tag='var')
                nc.vector.tensor_scalar(
                    out=var[:, :g], in0=a1[:, :g], scalar1=1.0 / 96.0,
                    scalar2=None, op0=ALU.mult)
                nc.vector.tensor_scalar(
                    out=a3[:, :g], in0=a3[:, :g], scalar1=0.25,
                    scalar2=None, op0=ALU.mult)
                nc.vector.tensor_add(
                    out=var[:, :g], in0=var[:, :g], in1=a3[:, :g])
                std = grp.tile([P, DG], f32, tag='std')
                nc.scalar.activation(
                    out=std[:, :g], in_=var[:, :g], func=AF.Sqrt,
                    bias=eps_col[:])
                rstd = grp.tile([P, DG], f32, tag='rstd')
                nc.vector.reciprocal(out=rstd[:, :g], in_=std[:, :g])
                msum = grp.tile([P, DG], f32, tag='msum')
                nc.vector.tensor_add(
                    out=msum[:, :g], in0=stv[:, :g, 1], in1=stv[:, :g, 4])
                nmr = grp.tile([P, DG], f32, tag='nmr')
                nc.vector.tensor_mul(
                    out=nmr[:, :g], in0=msum[:, :g], in1=rstd[:, :g])
                nc.vector.tensor_scalar(
                    out=nmr[:, :g], in0=nmr[:, :g], scalar1=-0.5,
                    scalar2=None, op0=ALU.mult)
                return rstd, nmr

            def flush():
                if not pending:
                    return
                g = pend_n[0]
                rstd, nmr = ln_coeffs(stats_buf[0], g)
                ybuf = ybp.tile([P, DG * H], bf16, tag='ybuf')
                j = 0
                off0 = pending[0][0]
                for (off_, t_, msb_, efg_) in pending:
                    for k in range(t_):
                        nc.scalar.activation(
                            out=ybuf[:, (j + k) * H:(j + k + 1) * H],
                            in_=msb_[:, k * H:(k + 1) * H],
                            func=AF.Silu,
                            bias=nmr[:, j + k:j + k + 1],
                            scale=rstd[:, j + k:j + k + 1])
                    nc.vector.tensor_add(
                        out=ybuf[:, j * H:(j + t_) * H],
                        in0=ybuf[:, j * H:(j + t_) * H],
                        in1=efg_)
                    j += t_
                nc.sync.dma_start(
                    out=y_pm[:, off0 * H:(off0 + g) * H],
                    in_=ybuf[:, :g * H])
                pending.clear()
                pend_n[0] = 0
                stats_buf[0] = None

            cur_key = None
            cur_span = None
            cur_s8 = [-1]
            t2base = 0
            sums = None
            sw_start = [False]
            win_base = [None]
            win_len = [0]
            win_eftg = [None]
            win_efg = [None]
            for (s, w, t, off, ni) in sched:
                if pend_n[0] + t > DG:
                    flush()
                if (s, w) != cur_key:
                    if cur_span is None or s // 8 != cur_s8[0]:
                        t2span = iop.tile([P, 8 * 192], bf16, tag='t2span')
                        s8 = (s // 8) * 8
                        cur_s8[0] = s // 8
                        sn = min(8, S - s8)
                        nc.scalar.dma_start(
                            out=t2span[:, :sn * 192],
                            in_=t2x[:, s8 * 192:(s8 + sn) * 192])
                        cur_span = t2span
                    cur_key = (s, w)
                    t2base = (s % 8) * 192
                    sums = pps.tile([P, 192], f32, space='PSUM', tag='sums')
                    sw_start[0] = True
                gbuf = gring[gring_i[0] % 5]
                gring_i[0] += 1
                nc.gpsimd.dma_gather(
                    out_ap=gbuf[:, :t * 256].rearrange(
                        'p (t d) -> p t d', t=t),
                    in_ap=t1cw[w][:],
                    idxs_ap=idx_all[:, off * 8:off * 8 + (ni + 15) // 16],
                    num_idxs=ni,
                    num_idxs_reg=ni,
                    elem_size=256,
                    single_packet=(ni <= 512),
                )
                if win_base[0] is None or off >= win_base[0] + win_len[0]:
                    wb = off
                    wl = 0
                    for (s2, w2, t2, off2, ni2) in sched:
                        if off2 < wb:
                            continue
                        if wl + t2 > DG:
                            break
                        wl += t2
                    win_base[0] = wb
                    win_len[0] = wl
                    eftg_w = efp.tile([H, DG * P], bf16, tag='eftgw')
                    nc.scalar.dma_start(
                        out=eftg_w[:, :wl * P],
                        in_=efT[:, wb * P:(wb + wl) * P])
                    efg_w = efp.tile([P, DG * H], bf16, tag='efgw')
                    nc.scalar.dma_start(
                        out=efg_w[:, :wl * H],
                        in_=ef_pm[:, wb * H:(wb + wl) * H])
                    win_eftg[0] = eftg_w
                    win_efg[0] = efg_w
                lo = off - win_base[0]

                # batched one-hot for the group
                onehot = wk.tile([P, t * P], bf16, tag='onehot')
                nc.vector.tensor_tensor(
                    out=onehot[:].rearrange('p (t q) -> p t q', q=P),
                    in0=dl_all[:, off:off + t, None].to_broadcast([P, t, P]),
                    in1=iota_sb[:, None, :].to_broadcast([P, t, P]),
                    op=ALU.is_equal)
                trps = ppt.tile([P, t * P], bf16, space='PSUM', tag='tr')
                for k in range(t):
                    nc.tensor.transpose(
                        out=trps[:, k * P:(k + 1) * P],
                        in_=onehot[:, k * P:(k + 1) * P],
                        identity=id_bf[:])
                ohne = wk.tile([P, t * P], bf16, tag='ohne')
                nc.vector.tensor_copy(out=ohne[:], in_=trps[:])

                mp = pp.tile([P, t * H], f32, space='PSUM', tag='mm')
                for k in range(t):
                    nc.tensor.matmul(
                        out=mp[:, k * H:(k + 1) * H],
                        lhsT=win_eftg[0][:, (lo + k) * P:(lo + k + 1) * P],
                        rhs=weg_sb[:], start=True, stop=False)
                    nc.tensor.matmul(
                        out=mp[:, k * H:(k + 1) * H],
                        lhsT=ohne[:, k * P:(k + 1) * P],
                        rhs=cur_span[:, t2base:t2base + H],
                        start=False, stop=True)
                msb = msp.tile([P, t * H], f32, tag='msb')
                nc.vector.tensor_add(
                    out=msb[:].rearrange('p (t f) -> p t f', f=H),
                    in0=mp[:].rearrange('p (t f) -> p t f', f=H),
                    in1=gbuf[:, :t * 256].rearrange('p (t d) -> p t d', d=256)[:, :, 0:H])

                valcat = wk.tile([P, t * 192], bf16, tag='valcat')
                vv = valcat[:].rearrange('p (t d) -> p t d', d=192)
                nc.scalar.activation(
                    out=vv[:, :, 0:H],
                    in_=msb[:].rearrange('p (t f) -> p t f', f=H),
                    func=AF.Sigmoid)
                nc.vector.tensor_tensor(
                    out=vv[:, :, H:192],
                    in0=gbuf[:, :t * 256].rearrange('p (t d) -> p t d', d=256)[:, :, H:192],
                    in1=vv[:, :, 0:H], op=ALU.mult)

                for k in range(t):
                    tt = off + k
                    nc.tensor.matmul(
                        out=sums[:],
                        lhsT=onehot[:, k * P:(k + 1) * P],
                        rhs=valcat[:, k * 192:(k + 1) * 192],
                        start=sw_start[0],
                        stop=(tt == sw_last[(s, w)]))
                    sw_start[0] = False

                if stats_buf[0] is None:
                    st_new = grp.tile([P, DG * 6], f32, tag='stats')
                    stats_buf[0] = st_new
                j = pend_n[0]
                for k0 in range(t):
                    nc.vector.bn_stats(
                        out=stats_buf[0][:, (j + k0) * 6:(j + k0 + 1) * 6],
                        in_=msb[:, k0 * H:(k0 + 1) * H])
                pending.append((off, t, msb,
                                win_efg[0][:, lo * H:(lo + t) * H]))
                pend_n[0] = j + t

                if off + t - 1 == sw_last[(s, w)]:
                    if w < last_w[s]:
                        # stash partial sums (bf16) for later windows
                        nc.vector.tensor_copy(
                            out=acc[:, s * 192:(s + 1) * 192], in_=sums[:])
                    else:
                        if sw_haveprev[(s, w)]:
                            tot = wk.tile([P, 192], f32, tag='tot')
                            nc.vector.tensor_add(
                                out=tot[:], in0=sums[:],
                                in1=acc[:, s * 192:(s + 1) * 192])
                            ss_ap, ssh_ap = tot[:, 0:H], tot[:, H:192]
                        else:
                            ss_ap, ssh_ap = sums[:, 0:H], sums[:, H:192]
                        ssd = wk.tile([P, H], f32, tag='ssd')
                        nc.scalar.activation(
                            out=ssd[:], in_=ss_ap, func=AF.Identity,
                            bias=eps6_col[:])
                        rec = wk.tile([P, H], f32, tag='rec')
                        nc.vector.reciprocal(out=rec[:], in_=ssd[:])
                        h = wk.tile([P, H], f32, tag='h')
                        nc.vector.tensor_mul(
                            out=h[:], in0=ssh_ap, in1=rec[:])
                        nc.vector.tensor_add(
                            out=acc[:, s * 192:s * 192 + H],
                            in0=h[:], in1=cur_span[:, t2base + H:t2base + 192])
            flush()

            # ---- final phase: node LN + silu + residual ----
            FG = 16
            for s0 in range(0, S, FG):
                g = min(FG, S - s0)
                st = grp.tile([P, DG * 6], f32, tag='stats')
                for k0 in range(g):
                    nc.vector.bn_stats(
                        out=st[:, k0 * 6:(k0 + 1) * 6],
                        in_=acc[:, (s0 + k0) * 192:(s0 + k0) * 192 + H])
                rstd, nmr = ln_coeffs(st, g)
                nfblk = ybp.tile([P, FG * H], f32, tag='nfblk')
                nc.sync.dma_start(
                    out=nfblk[:, :g * H],
                    in_=nfb[:, s0 * H:(s0 + g) * H])
                xbuf = ybp.tile([P, FG * H], f32, tag='xbuf')
                for k in range(g):
                    s = s0 + k
                    nc.scalar.activation(
                        out=xbuf[:, k * H:(k + 1) * H],
                        in_=acc[:, s * 192:s * 192 + H],
                        func=AF.Silu, bias=nmr[:, k:k + 1],
                        scale=rstd[:, k:k + 1])
                nc.vector.tensor_add(
                    out=xbuf[:, :g * H], in0=xbuf[:, :g * H],
                    in1=nfblk[:, :g * H])
                nc.sync.dma_start(
                    out=xout[:, s0 * H:(s0 + g) * H],
                    in_=xbuf[:, :g * H])

    nc.finalize()
    return nc


# ----------------------------------------------------------------------------
# top-level
# ----------------------------------------------------------------------------

_TRACE = [False]


def kernel(**inputs):
    from concourse.bass_utils import run_bass_kernel_spmd

    src = np.asarray(inputs['src'])
    dst = np.asarray(inputs['dst'])
    node_feats = np.asarray(inputs['node_feats'], np.float32)
    edge_feats = np.asarray(inputs['edge_feats'], np.float32)
    N, E = node_feats.shape[0], edge_feats.shape[0]

    plan = build_plan(src, dst, N)
    in_maps = build_inputs(plan, inputs)
    nc = build_kernel(plan)
    res = run_bass_kernel_spmd(
        nc, in_maps, core_ids=list(range(NCORES)), trace=_TRACE[0])
    kernel.last_result = res

    x = np.zeros((N, H), np.float32)
    y = np.zeros((E, H), np.float32)
    for c in range(NCORES):
        out = res.results[c]
        blocks = plan['slot_block'][c]
        xs = out['xout'].reshape(P, plan['S'], H).transpose(1, 0, 2)
        for s_i, b in enumerate(blocks):
            lo = b * P
            hi = min(lo + P, N)
            if lo < N:
                x[lo:hi] = xs[s_i, :hi - lo]
        canon = plan['canon_edge'][c]
        real = canon >= 0
        y_can = np.asarray(out['y_pm']).reshape(
            P, plan['TT'], H).transpose(1, 0, 2).reshape(plan['E_pad'], H)
        y[canon[real]] = y_can[real].astype(np.float32)
    return x, y


# revision 41
# speedup vs baseline: 1.1424x; 1.0440x over previous
"""ALIGNN edge-gated message passing on 8 Trainium2 NeuronCores.

Strategy: edges partitioned by dst-block across cores (no collectives).
Each core receives host-prepared, per-core data:
  - a compacted node table (only nodes referenced as src by its edges)
  - its own 98 node-blocks (128 nodes each) in load-balanced "slot" order
  - its edge slice in canonical (slot, window, tile) order
Device work per core:
  phase A : PE matmuls node_feats -> T1=[e_src+bias | Bh] (f32, compact rows)
            and T2x=[e_dst | x_lin] (bf16, own blocks)
  phase B : per (slot,window) group of 128-edge tiles: dma_gather T1 rows by
            src (int16 windows), one-hot(dst_local) via batched is_equal,
            m = ef@W_eg + onehot.T@T2blk (+ gathered), sigma = sigmoid(m),
            segment-sum via one-hot matmul accumulation in PSUM;
            LN sqrt batched over deferred groups of 32 tiles, then
            y = ef + silu(LN(m)) written edge-major
  phase D : per slot: xpre = x_lin + ssh/(ss+1e-6); final phase applies
            LN + silu + residual for all slots with batched stats
"""
import sys

if '/opt/trn_rl_repo' not in sys.path:
    sys.path.insert(0, '/opt/trn_rl_repo')

import numpy as np
import ml_dtypes

BF16 = ml_dtypes.bfloat16
H = 96
LN_EPS = 1e-5
NCORES = 8
P = 128
WMAX = 32640          # gather window rows (<= int16 max, mult of 128)
DG = 24               # deferred-LN group size (tiles)


# ----------------------------------------------------------------------------
# host-side plan
# ----------------------------------------------------------------------------

def build_plan(src, dst, N):
    E = src.shape[0]
    n_blocks_real = (N + P - 1) // P
    n_blocks = ((n_blocks_real + NCORES - 1) // NCORES) * NCORES
    S = n_blocks // NCORES              # slots per core
    N_pad = n_blocks * P

    blk_of_edge = dst // P
    blk_counts = np.bincount(blk_of_edge, minlength=n_blocks)

    # balanced assignment: sort blocks by count desc, greedily fill cores
    order = np.argsort(-blk_counts, kind='stable')
    core_load = np.zeros(NCORES, dtype=np.int64)
    core_nblk = np.zeros(NCORES, dtype=np.int64)
    blk_core = np.zeros(n_blocks, dtype=np.int64)
    for b in order:
        cands = np.where(core_nblk < S)[0]
        c = cands[np.argmin(core_load[cands])]
        blk_core[b] = c
        core_load[c] += blk_counts[b]
        core_nblk[c] += 1

    # per-core slot order: blocks sorted by count desc
    slot_block = np.zeros((NCORES, S), dtype=np.int64)   # slot -> block id
    for c in range(NCORES):
        blks = np.where(blk_core == c)[0]
        blks = blks[np.argsort(-blk_counts[blks], kind='stable')]
        slot_block[c] = blks

    edge_core = blk_core[blk_of_edge]
    slot_of_block = np.zeros(n_blocks, dtype=np.int64)
    for c in range(NCORES):
        slot_of_block[slot_block[c]] = np.arange(S)
    edge_slot = slot_of_block[blk_of_edge]

    # compacted src table per core
    srclist = []
    for c in range(NCORES):
        u = np.unique(src[edge_core == c])
        srclist.append(u)
    E_TBL = ((max(len(u) for u in srclist) + P - 1) // P) * P
    n_win = max(1, (E_TBL + WMAX - 1) // WMAX)
    WSZ = ((E_TBL // n_win + P - 1) // P) * P
    assert WSZ <= 32767

    src_pos = np.zeros((NCORES, E), dtype=np.int64)
    for c in range(NCORES):
        m = edge_core == c
        src_pos[c, m] = np.searchsorted(srclist[c], src[m])

    cnt = np.zeros((NCORES, S, n_win), dtype=np.int64)
    for c in range(NCORES):
        m = edge_core == c
        w = src_pos[c, m] // WSZ
        np.add.at(cnt[c], (edge_slot[m], w), 1)
    tiles_sw = np.maximum(np.ceil(cnt / P).astype(np.int64).max(axis=0), 0)
    tiles_sw[:, 0] = np.maximum(tiles_sw[:, 0], 1)   # every slot >=1 tile
    TT = int(tiles_sw.sum())
    E_pad = TT * P

    sched = []
    off = 0
    TMAXG = 5
    for w in range(n_win):
        for s in range(S):
            t = int(tiles_sw[s, w])
            base = 0
            while t > 0:
                tc_ = min(t, TMAXG)
                # real edges of this call across cores
                mx = 0
                for c in range(NCORES):
                    rc = min(max(cnt[c, s, w] - base * P, 0), tc_ * P)
                    mx = max(mx, int(rc))
                ni = min(((mx + 15) // 16) * 16, tc_ * P)
                ni = max(ni, (tc_ - 1) * P + 16)
                sched.append((s, w, tc_, off, ni))
                off += tc_
                t -= tc_
                base += tc_

    canon_edge = np.full((NCORES, E_pad), -1, dtype=np.int64)
    for c in range(NCORES):
        m = np.where(edge_core == c)[0]
        w = src_pos[c, m] // WSZ
        key = w * S + edge_slot[m]
        ordr = np.argsort(key, kind='stable')
        me, ke = m[ordr], key[ordr]
        group_off = {}
        for (s_, w_, t_, o_, ni_) in sched:
            if (s_, w_) not in group_off:
                group_off[(s_, w_)] = o_ * P
        pos = np.zeros(len(me), dtype=np.int64)
        start = 0
        for k in np.unique(ke):
            cnt_k = int((ke == k).sum())
            w_, s_ = divmod(int(k), S)
            base = group_off[(s_, w_)]
            pos[start:start + cnt_k] = base + np.arange(cnt_k)
            start += cnt_k
        canon_edge[c, pos] = me
    return dict(
        N_pad=N_pad, n_blocks=n_blocks, S=S, E_TBL=E_TBL, n_win=n_win,
        WSZ=WSZ, TT=TT, E_pad=E_pad, sched=sched, slot_block=slot_block,
        srclist=srclist, src_pos=src_pos, canon_edge=canon_edge,
    )


def build_inputs(plan, inputs):
    node_feats = np.asarray(inputs['node_feats'], np.float32)
    edge_feats = np.asarray(inputs['edge_feats'], np.float32)
    src = np.asarray(inputs['src'])
    dst = np.asarray(inputs['dst'])
    N = node_feats.shape[0]

    tp = (np.asarray(inputs['time_feats'], np.float32) @
          np.asarray(inputs['W_tp'], np.float32) +
          np.asarray(inputs['b_tp'], np.float32))[0]
    bias_src = np.asarray(inputs['b_sg'], np.float32) + tp + \
        np.asarray(inputs['b_eg'], np.float32)

    W1b = np.concatenate([
        np.concatenate([inputs['W_sg'], inputs['W_du']], axis=1),
        np.concatenate([bias_src, inputs['b_du']])[None, :],
    ], axis=0).astype(np.float32)                      # [97, 192]
    W2b = np.concatenate([
        np.concatenate([inputs['W_dg'], inputs['W_su']], axis=1),
        np.concatenate([inputs['b_dg'], inputs['b_su']])[None, :],
    ], axis=0).astype(np.float32)                      # [97, 192]

    S, E_TBL, E_pad, TT = plan['S'], plan['E_TBL'], plan['E_pad'], plan['TT']
    nf_pad = np.zeros((plan['N_pad'], H), np.float32)
    nf_pad[:N] = node_feats

    iota = np.tile(np.arange(P, dtype=np.float32), (P, 1))
    ident = np.eye(P, dtype=np.float32)

    in_maps = []
    for c in range(NCORES):
        u = plan['srclist'][c]
        nftc = np.zeros((97, E_TBL), np.float32)
        nftc[:H, :len(u)] = node_feats[u].T
        nftc[96, :] = 1.0

        blocks = plan['slot_block'][c]
        own = nf_pad.reshape(-1, P, H)[blocks]          # [S, 128, 96]
        own_flat = own.reshape(S * P, H)
        nfbT = np.zeros((97, S * P), np.float32)
        nfbT[:H] = own_flat.T
        nfbT[96] = 1.0

        canon = plan['canon_edge'][c]
        real = canon >= 0
        ef_can = np.zeros((E_pad, H), np.float32)
        ef_can[real] = edge_feats[canon[real]]
        ef_pm = ef_can.reshape(TT, P, H).transpose(1, 0, 2).reshape(P, TT * H)
        nfb_pm = own.transpose(1, 0, 2).reshape(P, S * H)

        dstloc = np.full(E_pad, -1.0, np.float32)
        dstloc[real] = (dst[canon[real]] % P).astype(np.float32)
        dstloc = dstloc.reshape(TT, P).T.copy()         # [128, TT]

        gpos = np.zeros(E_pad, np.int64)
        gpos[real] = plan['src_pos'][c, canon[real]] % plan['WSZ']
        gidx = np.zeros((16, E_pad // 16), np.int16)
        idx_lin = np.arange(E_pad)
        gidx[idx_lin % 16, idx_lin // 16] = gpos.astype(np.int16)
        gidx = np.tile(gidx, (8, 1))                    # [128, E_pad/16]

        in_maps.append({
            'nftc': nftc.astype(BF16),
            'nfbT': nfbT.astype(BF16),
            'w1b': W1b.astype(BF16), 'w2b': W2b.astype(BF16),
            'weg': np.asarray(inputs['W_eg'], np.float32).astype(BF16),
            'efT': ef_can.T.astype(BF16).copy(),
            'ef_pm': ef_pm.astype(BF16),
            'dstloc': dstloc,
            'gidx': gidx,
            'iota': iota,
            'ident': ident.astype(BF16),
            'nfb': nfb_pm,
        })
    return in_maps


# ----------------------------------------------------------------------------
# device kernel
# ----------------------------------------------------------------------------

def build_kernel(plan):
    import concourse.bacc as bacc
    import concourse.bass as bass
    import concourse.mybir as mybir
    import concourse.tile as tile

    f32, bf16, i16 = mybir.dt.float32, mybir.dt.bfloat16, mybir.dt.int16
    AF = mybir.ActivationFunctionType
    ALU = mybir.AluOpType

    S, E_TBL, E_pad, TT = plan['S'], plan['E_TBL'], plan['E_pad'], plan['TT']
    n_win, WSZ = plan['n_win'], plan['WSZ']
    sched = plan['sched']
    NB = S * P

    nc = bacc.Bacc()
    dp = nc.declare_dram_parameter
    nftc = dp('nftc', [97, E_TBL], bf16, isOutput=False)
    nfbT = dp('nfbT', [97, NB], bf16, isOutput=False)
    w1b = dp('w1b', [97, 192], bf16, isOutput=False)
    w2b = dp('w2b', [97, 192], bf16, isOutput=False)
    weg = dp('weg', [H, H], bf16, isOutput=False)
    efT = dp('efT', [H, E_pad], bf16, isOutput=False)
    ef_pm = dp('ef_pm', [P, TT * H], bf16, isOutput=False)
    dstloc = dp('dstloc', [P, TT], f32, isOutput=False)
    gidx = dp('gidx', [P, E_pad // 16], i16, isOutput=False)
    iota = dp('iota', [P, P], f32, isOutput=False)
    ident = dp('ident', [P, P], bf16, isOutput=False)
    nfb = dp('nfb', [P, S * H], f32, isOutput=False)
    y_pm = dp('y_pm', [P, TT * H], bf16, isOutput=True)
    xout = dp('xout', [P, S * H], f32, isOutput=True)

    t1cw = []
    for w in range(n_win):
        wr = min(WSZ, E_TBL - w * WSZ)
        t1cw.append(nc.dram_tensor(f't1c{w}', [wr, 256], bf16))
    t2x = nc.dram_tensor('t2x', [P, S * 192], bf16)

    with tile.TileContext(nc) as tc:
        with (
            tc.tile_pool(name='const', bufs=1) as cpool,
            tc.tile_pool(name='io', bufs=2) as iop,
            tc.tile_pool(name='pa', bufs=2) as pa,
            tc.tile_pool(name='eft', bufs=2) as efp,
            tc.tile_pool(name='msb', bufs=7) as msp,
            tc.tile_pool(name='work', bufs=3) as wk,
            tc.tile_pool(name='grp', bufs=2) as grp,
            tc.tile_pool(name='yb', bufs=2) as ybp,
            tc.tile_pool(name='ps', bufs=3, space='PSUM') as pp,
            tc.tile_pool(name='pst', bufs=2, space='PSUM') as ppt,
            tc.tile_pool(name='psa', bufs=1, space='PSUM') as ppa,
            tc.tile_pool(name='ps_sum', bufs=2, space='PSUM') as pps,
        ):
            # ---- constants ----
            iota_sb = cpool.tile([P, P], f32, tag='iota')
            nc.sync.dma_start(out=iota_sb[:], in_=iota[:])
            id_bf = cpool.tile([P, P], bf16, tag='idb')
            nc.sync.dma_start(out=id_bf[:], in_=ident[:])
            w1_sb = cpool.tile([97, 192], bf16, tag='w1')
            nc.sync.dma_start(out=w1_sb[:], in_=w1b[:])
            w2_sb = cpool.tile([97, 192], bf16, tag='w2')
            nc.sync.dma_start(out=w2_sb[:], in_=w2b[:])
            weg_sb = cpool.tile([H, H], bf16, tag='weg')
            nc.sync.dma_start(out=weg_sb[:], in_=weg[:])
            idx_all = cpool.tile([P, E_pad // 16], i16, tag='gidx')
            nc.sync.dma_start(out=idx_all[:], in_=gidx[:])
            dl_all = cpool.tile([P, TT], f32, tag='dstloc')
            nc.sync.dma_start(out=dl_all[:], in_=dstloc[:])
            eps_col = cpool.tile([P, 1], f32, tag='eps')
            nc.vector.memset(eps_col[:], LN_EPS)
            eps6_col = cpool.tile([P, 1], f32, tag='eps6')
            nc.vector.memset(eps6_col[:], 1e-6)
            # acc[s]: bf16 partial sums per slot (windows < last); after
            # finalize the first 96 cols hold xpre for the final phase
            acc = cpool.tile([P, S * 192], bf16, tag='acc')

            # ---- phase A: node transform tables ----
            ACH = 16
            phase_a = []
            for w in range(n_win):
                wr = min(WSZ, E_TBL - w * WSZ)
                phase_a.append(
                    ('t1', nftc, w1_sb, w * WSZ // P, wr // P, 256, w))
            phase_a.insert(1, ('t2', nfbT, w2_sb, 0, S, 192, None))
            naring, tbring = [], []
            for r in range(3):
                nt = cpool.tile([97, ACH * P], bf16, tag=f'nfa{r}')
                naring.append(nt)
                tb = cpool.tile([P, ACH * 256], bf16, tag=f'tb{r}')
                nc.vector.memset(tb[:], 0)
                tbring.append(tb)
            ring_j = [0]
            for (mode, srcT, wsb, tile0, n_tiles, dcols, wid) in phase_a:
                for j0 in range(0, n_tiles, ACH):
                    jn = min(ACH, n_tiles - j0)
                    nchunk = naring[ring_j[0] % 3]
                    tbuf = tbring[ring_j[0] % 3]
                    ring_j[0] += 1
                    nc.scalar.dma_start(
                        out=nchunk[:, :jn * P],
                        in_=srcT[:, (tile0 + j0) * P:(tile0 + j0 + jn) * P])
                    for k in range(0, jn, 2):
                        kn = min(2, jn - k)
                        mm = ppa.tile([P, 2 * 192], f32, space='PSUM',
                                      tag='pamm')
                        for q in range(kn):
                            nc.tensor.matmul(
                                out=mm[:, q * 192:(q + 1) * 192],
                                lhsT=nchunk[:, (k + q) * P:(k + q + 1) * P],
                                rhs=wsb[:], start=True, stop=True)
                        nc.vector.tensor_copy(
                            out=tbuf[:, k * dcols:k * dcols + kn * dcols]
                            .rearrange('p (j d) -> p j d', d=dcols)[:, :, 0:192]
                            if dcols == 256 else
                            tbuf[:, k * dcols:(k + kn) * dcols],
                            in_=mm[:, :kn * 192].rearrange(
                                'p (j d) -> p j d', d=192)
                            if dcols == 256 else mm[:, :kn * 192])
                    if mode == 't1':
                        nc.sync.dma_start(
                            out=t1cw[wid][j0 * P:(j0 + jn) * P, :].rearrange(
                                '(j p) d -> p j d', p=P),
                            in_=tbuf[:, :jn * 256].rearrange(
                                'p (j d) -> p j d', d=256))
                    else:
                        nc.sync.dma_start(
                            out=t2x[:, j0 * 192:(j0 + jn) * 192],
                            in_=tbuf[:, :jn * 192])

            # ---- phase B ----
            sw_last = {}
            sw_haveprev = {}
            for (s, w, t, off, ni) in sched:
                sw_last[(s, w)] = off + t - 1
                sw_haveprev[s] = {}
            last_w = {}
            for (s, w) in sw_last:
                last_w[s] = max(last_w.get(s, 0), w)
            seen_w = {}
            for (s, w) in sorted(sw_last):
                sw_haveprev[(s, w)] = any(
                    (s, w2) in sw_last for w2 in range(w))
            # DIY gather ring (stale-safe: memset once)
            TMAXG = 5
            gring = []
            for r in range(5):
                gt = cpool.tile([P, TMAXG * 256], bf16, tag=f'gring{r}')
                nc.vector.memset(gt[:], 0)
                gring.append(gt)
            gring_i = [0]

            pending = []     # (off, t, msb, efg) per (s,w) group
            pend_n = [0]
            stats_buf = [None]

            def ln_coeffs(st, g):
                """Batched LN: stats [P, g, 6] -> (rstd, nmr) [P, g]."""
                stv = st[:].rearrange('p (g s) -> p g s', s=6)
                a1 = grp.tile([P, DG], f32, tag='a1')
                nc.vector.tensor_add(
                    out=a1[:, :g], in0=stv[:, :g, 2], in1=stv[:, :g, 5])
                a2 = grp.tile([P, DG], f32, tag='a2')
                nc.vector.tensor_sub(
                    out=a2[:, :g], in0=stv[:, :g, 1], in1=stv[:, :g, 4])
                a3 = grp.tile([P, DG], f32, tag='a3')
                nc.vector.tensor_mul(
                    out=a3[:, :g], in0=a2[:, :g], in1=a2[:, :g])
                var = grp.tile([P, DG], f32, tag='var')
                nc.vector.tensor_scalar(
                    out=var[:, :g], in0=a1[:, :g], scalar1=1.0 / 96.0,
                    scalar2=None, op0=ALU.mult)
                nc.vector.tensor_scalar(
                    out=a3[:, :g], in0=a3[:, :g], scalar1=0.25,
                    scalar2=None, op0=ALU.mult)
                nc.vector.tensor_add(
                    out=var[:, :g], in0=var[:, :g], in1=a3[:, :g])
                std = grp.tile([P, DG], f32, tag='std')
                nc.scalar.activation(
                    out=std[:, :g], in_=var[:, :g], func=AF.Sqrt,
                    bias=eps_col[:])
                rstd = grp.tile([P, DG], f32, tag='rstd')
                nc.vector.reciprocal(out=rstd[:, :g], in_=std[:, :g])
                msum = grp.tile([P, DG], f32, tag='msum')
                nc.vector.tensor_add(
                    out=msum[:, :g], in0=stv[:, :g, 1], in1=stv[:, :g, 4])
                nmr = grp.tile([P, DG], f32, tag='nmr')
                nc.vector.tensor_mul(
                    out=nmr[:, :g], in0=msum[:, :g], in1=rstd[:, :g])
                nc.vector.tensor_scalar(
                    out=nmr[:, :g], in0=nmr[:, :g], scalar1=-0.5,
                    scalar2=None, op0=ALU.mult)
                return rstd, nmr

            def flush():
                if not pending:
                    return
                g = pend_n[0]
                rstd, nmr = ln_coeffs(stats_buf[0], g)
                ybuf = ybp.tile([P, DG * H], bf16, tag='ybuf')
                j = 0
                off0 = pending[0][0]
                for (off_, t_, msb_, efg_) in pending:
                    for k in range(t_):
                        nc.scalar.activation(
                            out=ybuf[:, (j + k) * H:(j + k + 1) * H],
                            in_=msb_[:, k * H:(k + 1) * H],
                            func=AF.Silu,
                            bias=nmr[:, j + k:j + k + 1],
                            scale=rstd[:, j + k:j + k + 1])
                    nc.vector.tensor_add(
                        out=ybuf[:, j * H:(j + t_) * H],
                        in0=ybuf[:, j * H:(j + t_) * H],
                        in1=efg_)
                    j += t_
                nc.sync.dma_start(
                    out=y_pm[:, off0 * H:(off0 + g) * H],
                    in_=ybuf[:, :g * H])
                pending.clear()
                pend_n[0] = 0
                stats_buf[0] = None

            cur_key = None
            cur_span = None
            cur_s8 = [-1]
            t2base = 0
            sums = None
            sw_start = [False]
            win_base = [None]
            win_len = [0]
            win_eftg = [None]
            win_efg = [None]
            for (s, w, t, off, ni) in sched:
                if pend_n[0] + t > DG:
                    flush()
                if (s, w) != cur_key:
                    if cur_span is None or s // 8 != cur_s8[0]:
                        t2span = iop.tile([P, 8 * 192], bf16, tag='t2span')
                        s8 = (s // 8) * 8
                        cur_s8[0] = s // 8
                        sn = min(8, S - s8)
                        nc.sync.dma_start(
                            out=t2span[:, :sn * 192],
                            in_=t2x[:, s8 * 192:(s8 + sn) * 192])
                        cur_span = t2span
                    cur_key = (s, w)
                    t2base = (s % 8) * 192
                    sums = pps.tile([P, 192], f32, space='PSUM', tag='sums')
                    sw_start[0] = True
                gbuf = gring[gring_i[0] % 5]
                gring_i[0] += 1
                nc.gpsimd.dma_gather(
                    out_ap=gbuf[:, :t * 256].rearrange(
                        'p (t d) -> p t d', t=t),
                    in_ap=t1cw[w][:],
                    idxs_ap=idx_all[:, off * 8:off * 8 + (ni + 15) // 16],
                    num_idxs=ni,
                    num_idxs_reg=ni,
                    elem_size=256,
                    single_packet=(ni <= 512),
                )
                if win_base[0] is None or off >= win_base[0] + win_len[0]:
                    wb = off
                    wl = 0
                    for (s2, w2, t2, off2, ni2) in sched:
                        if off2 < wb:
                            continue
                        if wl + t2 > DG:
                            break
                        wl += t2
                    win_base[0] = wb
                    win_len[0] = wl
                    eftg_w = efp.tile([H, DG * P], bf16, tag='eftgw')
                    nc.sync.dma_start(
                        out=eftg_w[:, :wl * P],
                        in_=efT[:, wb * P:(wb + wl) * P])
                    efg_w = efp.tile([P, DG * H], bf16, tag='efgw')
                    nc.sync.dma_start(
                        out=efg_w[:, :wl * H],
                        in_=ef_pm[:, wb * H:(wb + wl) * H])
                    win_eftg[0] = eftg_w
                    win_efg[0] = efg_w
                lo = off - win_base[0]

                # batched one-hot for the group
                onehot = wk.tile([P, t * P], bf16, tag='onehot')
                nc.vector.tensor_tensor(
                    out=onehot[:].rearrange('p (t q) -> p t q', q=P),
                    in0=dl_all[:, off:off + t, None].to_broadcast([P, t, P]),
                    in1=iota_sb[:, None, :].to_broadcast([P, t, P]),
                    op=ALU.is_equal)
                trps = ppt.tile([P, t * P], bf16, space='PSUM', tag='tr')
                for k in range(t):
                    nc.tensor.transpose(
                        out=trps[:, k * P:(k + 1) * P],
                        in_=onehot[:, k * P:(k + 1) * P],
                        identity=id_bf[:])
                ohne = wk.tile([P, t * P], bf16, tag='ohne')
                nc.vector.tensor_copy(out=ohne[:], in_=trps[:])

                mp = pp.tile([P, t * H], f32, space='PSUM', tag='mm')
                for k in range(t):
                    nc.tensor.matmul(
                        out=mp[:, k * H:(k + 1) * H],
                        lhsT=win_eftg[0][:, (lo + k) * P:(lo + k + 1) * P],
                        rhs=weg_sb[:], start=True, stop=False)
                    nc.tensor.matmul(
                        out=mp[:, k * H:(k + 1) * H],
                        lhsT=ohne[:, k * P:(k + 1) * P],
                        rhs=cur_span[:, t2base:t2base + H],
                        start=False, stop=True)
                msb = msp.tile([P, t * H], f32, tag='msb')
                nc.vector.tensor_add(
                    out=msb[:].rearrange('p (t f) -> p t f', f=H),
                    in0=mp[:].rearrange('p (t f) -> p t f', f=H),
                    in1=gbuf[:, :t * 256].rearrange('p (t d) -> p t d', d=256)[:, :, 0:H])

                valcat = wk.tile([P, t * 192], bf16, tag='valcat')
                vv = valcat[:].rearrange('p (t d) -> p t d', d=192)
                nc.scalar.activation(
                    out=vv[:, :, 0:H],
                    in_=msb[:].rearrange('p (t f) -> p t f', f=H),
                    func=AF.Sigmoid)
                nc.vector.tensor_tensor(
                    out=vv[:, :, H:192],
                    in0=gbuf[:, :t * 256].rearrange('p (t d) -> p t d', d=256)[:, :, H:192],
                    in1=vv[:, :, 0:H], op=ALU.mult)

                for k in range(t):
                    tt = off + k
                    nc.tensor.matmul(
                        out=sums[:],
                        lhsT=onehot[:, k * P:(k + 1) * P],
                        rhs=valcat[:, k * 192:(k + 1) * 192],
                        start=sw_start[0],
                        stop=(tt == sw_last[(s, w)]))
                    sw_start[0] = False

                if stats_buf[0] is None:
                    st_new = grp.tile([P, DG * 6], f32, tag='stats')
                    stats_buf[0] = st_new
                j = pend_n[0]
                for k0 in range(t):
                    nc.vector.bn_stats(
                        out=stats_buf[0][:, (j + k0) * 6:(j + k0 + 1) * 6],
                        in_=msb[:, k0 * H:(k0 + 1) * H])
                pending.append((off, t, msb,
                                win_efg[0][:, lo * H:(lo + t) * H]))
                pend_n[0] = j + t

                if off + t - 1 == sw_last[(s, w)]:
                    if w < last_w[s]:
                        # stash partial sums (bf16) for later windows
                        nc.vector.tensor_copy(
                            out=acc[:, s * 192:(s + 1) * 192], in_=sums[:])
                    else:
                        if sw_haveprev[(s, w)]:
                            tot = wk.tile([P, 192], f32, tag='tot')
                            nc.vector.tensor_add(
                                out=tot[:], in0=sums[:],
                                in1=acc[:, s * 192:(s + 1) * 192])
                            ss_ap, ssh_ap = tot[:, 0:H], tot[:, H:192]
                        else:
                            ss_ap, ssh_ap = sums[:, 0:H], sums[:, H:192]
                        ssd = wk.tile([P, H], f32, tag='ssd')
                        nc.scalar.activation(
                            out=ssd[:], in_=ss_ap, func=AF.Identity,
                            bias=eps6_col[:])
                        rec = wk.tile([P, H], f32, tag='rec')
                        nc.vector.reciprocal(out=rec[:], in_=ssd[:])
                        h = wk.tile([P, H], f32, tag='h')
                        nc.vector.tensor_mul(
                            out=h[:], in0=ssh_ap, in1=rec[:])
                        nc.vector.tensor_add(
                            out=acc[:, s * 192:s * 192 + H],
                            in0=h[:], in1=cur_span[:, t2base + H:t2base + 192])
            flush()

            # ---- final phase: node LN + silu + residual ----
            FG = 16
            for s0 in range(0, S, FG):
                g = min(FG, S - s0)
                st = grp.tile([P, DG * 6], f32, tag='stats')
                for k0 in range(g):
                    nc.vector.bn_stats(
                        out=st[:, k0 * 6:(k0 + 1) * 6],
                        in_=acc[:, (s0 + k0) * 192:(s0 + k0) * 192 + H])
                rstd, nmr = ln_coeffs(st, g)
                nfblk = ybp.tile([P, FG * H], f32, tag='nfblk')
                nc.sync.dma_start(
                    out=nfblk[:, :g * H],
                    in_=nfb[:, s0 * H:(s0 + g) * H])
                xbuf = ybp.tile([P, FG * H], f32, tag='xbuf')
                for k in range(g):
                    s = s0 + k
                    nc.scalar.activation(
                        out=xbuf[:, k * H:(k + 1) * H],
                        in_=acc[:, s * 192:s * 192 + H],
                        func=AF.Silu, bias=nmr[:, k:k + 1],
                        scale=rstd[:, k:k + 1])
                nc.vector.tensor_add(
                    out=xbuf[:, :g * H], in0=xbuf[:, :g * H],
                    in1=nfblk[:, :g * H])
                nc.sync.dma_start(
                    out=xout[:, s0 * H:(s0 + g) * H],
                    in_=xbuf[:, :g * H])

    nc.finalize()
    return nc


# ----------------------------------------------------------------------------
# top-level
# ----------------------------------------------------------------------------

_TRACE = [False]


def kernel(**inputs):
    from concourse.bass_utils import run_bass_kernel_spmd

    src = np.asarray(inputs['src'])
    dst = np.asarray(inputs['dst'])
    node_feats = np.asarray(inputs['node_feats'], np.float32)
    edge_feats = np.asarray(inputs['edge_feats'], np.float32)
    N, E = node_feats.shape[0], edge_feats.shape[0]

    plan = build_plan(src, dst, N)
    in_maps = build_inputs(plan, inputs)
    nc = build_kernel(plan)
    res = run_bass_kernel_spmd(
        nc, in_maps, core_ids=list(range(NCORES)), trace=_TRACE[0])
    kernel.last_result = res

    x = np.zeros((N, H), np.float32)
    y = np.zeros((E, H), np.float32)
    for c in range(NCORES):
        out = res.results[c]
        blocks = plan['slot_block'][c]
        xs = out['xout'].reshape(P, plan['S'], H).transpose(1, 0, 2)
        for s_i, b in enumerate(blocks):
            lo = b * P
            hi = min(lo + P, N)
            if lo < N:
                x[lo:hi] = xs[s_i, :hi - lo]
        canon = plan['canon_edge'][c]
        real = canon >= 0
        y_can = np.asarray(out['y_pm']).reshape(
            P, plan['TT'], H).transpose(1, 0, 2).reshape(plan['E_pad'], H)
        y[canon[real]] = y_can[real].astype(np.float32)
    return x, y
